# revision 4
# baseline (speedup 1.0000x reference)
"""Trainium2 Bass kernel for nn_Encoder (FSPool set encoder), v2.

Computation per event b (8192 events, data-parallel over 8 cores):
  h = relu(x[b].reshape(128,4) @ W1 + b1)        # per-particle MLP
  h = relu(h @ W2 + b2)
  z = h @ W3 (+ b3)                              # [128 particles, 32 ch]
  z_sorted = sort_desc(z.T, axis=-1)             # per-channel sort over particles
  pooled[c] = sum_p z_sorted[c,p] * w[c,p]       # rank-weighted pool
  mus = pooled[::2]; logvars = pooled[1::2]
  samples = mus + eps * exp(0.5*logvars)

v2 engine assignment (per core, 1024 events):
  - PE: the three matmuls (hidden dim on partitions, particles streaming).
  - ACT: relu1/relu2 as [128,1024] ops (2-DG interleaved PSUM schedule) and
    the PSUM->SBUF slot-interleave copy of z (only ACT can read PSUM besides
    DVE, and DVE is the bottleneck).
  - DVE: ONLY the per-channel descending sort - Batcher merge-exchange
    (28 passes, 1471 comparators vs bitonic's 1792), fp16 2x mode, with
    untouched-run carry-copies riding the 4x TensorCopy mode.
  - Pool (GPSIMD): rank-weight multiply, log-tree fold reduction over
    particles, offset add, and the sampling epilogue (SBUF-only engine).
  - b3 never enters the sort: sorting z+const shifts all ranks equally, so
    its pooled contribution b3[c]*sum_p w[c,p] is a per-partition offset.
"""

import os
import numpy as np

NCORES = 8
B = 8192
P = 128          # particles per event (set size)
F = 4            # input features per particle
H = 128          # hidden width
C = 32           # 2*LATENT pooled channels
LAT = 16
NPIECES = 20

E = B // NCORES          # events per core
NG = 72                  # max slots (groups of 4 events) per supertile
GALL = E // 4            # total groups per core (stage columns)

# events per supertile: geometric ramp (each chunk's sort covers the next
# chunk's MLP: c[i+1] <= ~1.5*c[i]), tapered tail to shorten the drain
CHUNKS = [64, 96, 144, 192, 288, 176, 64]
assert sum(CHUNKS) == E and all(c % 16 == 0 for c in CHUNKS)

_BUILT = None
LAST_RESULTS = None      # test harness can inspect exec_time_ns / profile


def _oems_passes(n):
    """Batcher merge-exchange (Knuth Alg M): list of (d, p, r) passes.
    Pass = compare-exchange (i, i+d) for i in [0, n-d) with i & p == r,
    max written to i (descending sort)."""
    passes = []
    p = n // 2
    while p >= 1:
        q = n // 2
        r = 0
        d = p
        while True:
            passes.append((d, p, r))
            if q == p:
                break
            d = q - p
            q //= 2
            r = p
        p //= 2
    return passes


def _carry_sets(n, passes):
    """Parity-aware carry copies: a position untouched for an even number
    of consecutive passes lands back in the correct ping-pong buffer by
    itself; only odd-length untouched runs need one copy (emitted at the
    run's first pass). Returns per-pass position sets."""
    touched = []
    for (d, p, r) in passes:
        t = set()
        for i in range(n - d):
            if (i & p) == r:
                t.add(i)
                t.add(i + d)
        touched.append(t)
    copy_sets = [set() for _ in passes]
    for x in range(n):
        k = 0
        while k < len(passes):
            if x in touched[k]:
                k += 1
                continue
            j = k
            while j < len(passes) and x not in touched[j]:
                j += 1
            if (j - k) % 2 == 1:
                copy_sets[k].add(x)
            k = j
    return copy_sets


def _group_runs(positions):
    """Group a position set into uniformly-strided run groups
    (s0, stride, count, run_len)."""
    if not positions:
        return []
    pos = sorted(positions)
    runs = []
    i = 0
    while i < len(pos):
        j = i
        while j + 1 < len(pos) and pos[j + 1] == pos[j] + 1:
            j += 1
        runs.append((pos[i], j - i + 1))
        i = j + 1
    groups = []
    k = 0
    while k < len(runs):
        s0, ln = runs[k]
        m = 1
        if k + 1 < len(runs):
            stride = runs[k + 1][0] - s0
            while (k + m < len(runs) and runs[k + m][1] == ln
                   and runs[k + m][0] == s0 + m * stride):
                m += 1
        else:
            stride = ln
        groups.append((s0, stride if m > 1 else ln, m, ln))
        k += m
    return groups


def _fspool_interp_matrix():
    """M [21, 128] with w_table = pool_weight @ M (matches reference math)."""
    pos = (np.arange(P, dtype=np.float32) / np.float32(P - 1)) * np.float32(NPIECES)
    idx = np.clip(pos.astype(np.int32), 0, NPIECES)
    frac = pos - idx.astype(np.float32)
    M = np.zeros((NPIECES + 1, P), dtype=np.float32)
    for p in range(P):
        i = int(idx[p])
        M[i, p] += np.float32(1.0) - frac[p]
        M[min(i + 1, NPIECES), p] += frac[p]
    return M


def _build():
    global _BUILT
    if _BUILT is not None:
        return _BUILT
    from contextlib import ExitStack
    import concourse.bass as bass
    import concourse.bacc as bacc
    import concourse.tile as tile
    import concourse.mybir as mybir

    f32 = mybir.dt.float32
    f16 = mybir.dt.float16
    AF = mybir.ActivationFunctionType
    OP = mybir.AluOpType

    nc = bacc.Bacc("TRN2", target_bir_lowering=False, debug=False)

    xt_d = nc.dram_tensor("xt", [F, E * P], f16, kind="ExternalInput")
    w1_d = nc.dram_tensor("w1", [F, H], f16, kind="ExternalInput")
    w2_d = nc.dram_tensor("w2", [H, H], f16, kind="ExternalInput")
    w3_d = nc.dram_tensor("w3", [H, C], f16, kind="ExternalInput")
    b1_d = nc.dram_tensor("b1", [H, 1], f32, kind="ExternalInput")
    b2_d = nc.dram_tensor("b2", [H, 1], f32, kind="ExternalInput")
    wrept_d = nc.dram_tensor("wrept", [128, P, NG], f16, kind="ExternalInput")
    offs_d = nc.dram_tensor("offs", [128, 1], f32, kind="ExternalInput")
    epst_d = nc.dram_tensor("epst", [128, GALL], f32, kind="ExternalInput")

    stage_d = nc.dram_tensor("stage_t", [128, GALL], f32, kind="ExternalOutput")
    smp_d = nc.dram_tensor("samples_t", [128, GALL], f32, kind="ExternalOutput")

    PASSES = _oems_passes(P)
    CARRIES = [_group_runs(s) for s in _carry_sets(P, PASSES)]

    with tile.TileContext(nc) as tc:
        with ExitStack() as ctx:
            consts = ctx.enter_context(tc.tile_pool(name="consts", bufs=1))
            xpool = ctx.enter_context(tc.tile_pool(name="x", bufs=3))
            hpool = ctx.enter_context(tc.tile_pool(name="h", bufs=4))
            zapool = ctx.enter_context(tc.tile_pool(name="za", bufs=2))
            zbpool = ctx.enter_context(tc.tile_pool(name="zb", bufs=1))
            ppool = ctx.enter_context(tc.tile_pool(name="prodp", bufs=1))
            fpool = ctx.enter_context(tc.tile_pool(name="fold", bufs=1))
            spool = ctx.enter_context(tc.tile_pool(name="stage", bufs=1))
            epool = ctx.enter_context(tc.tile_pool(name="epi", bufs=2))
            psh = ctx.enter_context(tc.tile_pool(name="psh", bufs=3, space="PSUM"))
            psz = ctx.enter_context(tc.tile_pool(name="psz", bufs=2, space="PSUM"))

            w1_s = consts.tile([F, H], f16)
            nc.sync.dma_start(out=w1_s[:], in_=w1_d[:])
            b1_s = consts.tile([H, 1], f32)
            nc.sync.dma_start(out=b1_s[:], in_=b1_d[:])
            # w2/w3/b2 go on the Pool queue: SP stays clear for chunk-0's xt
            w2_s = consts.tile([H, H], f16)
            nc.gpsimd.dma_start(out=w2_s[:], in_=w2_d[:])
            w3_s = consts.tile([H, C], f16)
            nc.gpsimd.dma_start(out=w3_s[:], in_=w3_d[:])
            b2_s = consts.tile([H, 1], f32)
            nc.gpsimd.dma_start(out=b2_s[:], in_=b2_d[:])
            # wrept/offs/epst are not needed until the first pooling
            # (~45us in); keep their DMAs off the queue head so chunk-0's
            # xt transfer starts immediately.
            wrept_s = consts.tile([128, P, NG], f16)
            offs_s = consts.tile([128, 1], f32)
            epst_s = consts.tile([128, GALL], f32)
            late_consts = [False]

            stage = spool.tile([128, GALL], f32)

            def sview(t, s0, stride, m, ln, ns):
                """[128, m, ln, ns] view of particle runs {s0+j*stride+k},
                k<ln, over slot range [0, ns)."""
                if m == 1:
                    return t[:, s0:s0 + ln, 0:ns].rearrange(
                        "a (mm p) s -> a mm p s", mm=1)
                if s0 + m * stride <= P:
                    return t[:, s0:s0 + m * stride, 0:ns].rearrange(
                        "a (mm st) s -> a mm st s", st=stride)[:, :, 0:ln, :]
                s_adj = s0 - (stride - ln)
                assert s_adj >= 0 and s_adj + m * stride <= P
                return t[:, s_adj:s_adj + m * stride, 0:ns].rearrange(
                    "a (mm st) s -> a mm st s",
                    st=stride)[:, :, stride - ln:stride, :]

            ev0 = 0
            for st_i, st_e in enumerate(CHUNKS):
                ng = st_e // 4
                col0 = ev0 // 4
                zA = zapool.tile([128, P, NG], f16, tag="zA")
                zB = zbpool.tile([128, P, NG], f16, tag="zB")

                # ---- MLP: process DGs (8 events = 1024 particle-columns)
                # in pairs so relu ops are [128,1024] with 3 shared 2-bank
                # PSUM buffers; z accumulates in 1-bank psz tiles (16 ev).
                n_dg = st_e // 8
                xts = {}
                for x0 in range(0, st_e, 64):
                    xe = min(64, st_e - x0)
                    xt_s = xpool.tile([F, 64 * P], f16, tag="xt")
                    if st_i == 0:
                        # ramp: split the load so the first matmul starts
                        # as soon as the first 16 events land
                        for y0 in range(0, xe, 16):
                            ye = min(16, xe - y0)
                            nc.sync.dma_start(
                                out=xt_s[:, y0 * P:(y0 + ye) * P],
                                in_=xt_d[:, (ev0 + x0 + y0) * P:
                                         (ev0 + x0 + y0 + ye) * P])
                    else:
                        nc.sync.dma_start(
                            out=xt_s[:, 0:xe * P],
                            in_=xt_d[:, (ev0 + x0) * P:(ev0 + x0 + xe) * P])
                    xts[x0] = xt_s
                for dgq in range(0, n_dg, 2):
                    base = dgq * 8
                    xt_s = xts[(base // 64) * 64]
                    xo = (base % 64) * P
                    xta = xt_s[:, xo:xo + 1024]
                    xtb = xt_s[:, xo + 1024:xo + 2048]

                    ph1a = psh.tile([128, 1024], f32, tag="ph")
                    nc.tensor.matmul(ph1a[:, 0:512], w1_s[:], xta[:, 0:512],
                                     start=True, stop=True)
                    nc.tensor.matmul(ph1a[:, 512:1024], w1_s[:], xta[:, 512:1024],
                                     start=True, stop=True)
                    ph1b = psh.tile([128, 1024], f32, tag="ph")
                    nc.tensor.matmul(ph1b[:, 0:512], w1_s[:], xtb[:, 0:512],
                                     start=True, stop=True)
                    nc.tensor.matmul(ph1b[:, 512:1024], w1_s[:], xtb[:, 512:1024],
                                     start=True, stop=True)
                    def relu(dst, src, bias, on_dve=False):
                        if st_i == 0 and on_dve:
                            # pipeline ramp: DVE is idle until the first
                            # sort; run half of chunk-0's relus there (1x
                            # from PSUM) in parallel with ACT's half
                            nc.vector.tensor_scalar(
                                out=dst, in0=src, scalar1=bias,
                                scalar2=0.0, op0=OP.add, op1=OP.max)
                        else:
                            nc.scalar.activation(dst, src, AF.Relu, bias=bias)

                    h1a = hpool.tile([128, 1024], f16, tag="h1")
                    relu(h1a[:], ph1a[:], b1_s[:], on_dve=True)
                    h1b = hpool.tile([128, 1024], f16, tag="h1")
                    relu(h1b[:], ph1b[:], b1_s[:], on_dve=True)

                    ph2a = psh.tile([128, 1024], f32, tag="ph")
                    nc.tensor.matmul(ph2a[:, 0:512], w2_s[:], h1a[:, 0:512],
                                     start=True, stop=True)
                    nc.tensor.matmul(ph2a[:, 512:1024], w2_s[:], h1a[:, 512:1024],
                                     start=True, stop=True)
                    h2a = hpool.tile([128, 1024], f16, tag="h2")
                    relu(h2a[:], ph2a[:], b2_s[:], on_dve=True)
                    ph2b = psh.tile([128, 1024], f32, tag="ph")
                    nc.tensor.matmul(ph2b[:, 0:512], w2_s[:], h1b[:, 0:512],
                                     start=True, stop=True)
                    nc.tensor.matmul(ph2b[:, 512:1024], w2_s[:], h1b[:, 512:1024],
                                     start=True, stop=True)
                    h2b = hpool.tile([128, 1024], f16, tag="h2")
                    relu(h2b[:], ph2b[:], b2_s[:], on_dve=True)

                    # z for these 16 events -> one PSUM bank, then one ACT
                    # copy into the slot-interleaved sort tile (4 slots).
                    pz = psz.tile([128, 4 * P], f32, tag="pz")
                    for gg in range(4):
                        h2 = h2a if gg < 2 else h2b
                        off = (gg % 2) * 512
                        for e4 in range(4):
                            nc.tensor.matmul(
                                pz[32 * e4:32 * (e4 + 1), gg * P:(gg + 1) * P],
                                w3_s[:],
                                h2[:, off + e4 * P:off + (e4 + 1) * P],
                                start=True, stop=True,
                                tile_position=(0, 32 * e4),
                            )
                    s0 = dgq * 2
                    nc.scalar.activation(
                        zA[:, :, s0:s0 + 4].rearrange("a p s -> a s p"),
                        pz[:].rearrange("a (s p) -> a s p", p=P),
                        AF.Copy,
                    )

                # after chunk-0's MLP is on the queues, enqueue the consts
                # used by pooling/epilogue (needed from ~45us onward; kept
                # off the queue head so chunk-0's xt transfer goes first)
                if not late_consts[0]:
                    late_consts[0] = True
                    nc.sync.dma_start(out=wrept_s[:], in_=wrept_d[:])
                    nc.sync.dma_start(out=offs_s[:], in_=offs_d[:])
                    nc.sync.dma_start(out=epst_s[:], in_=epst_d[:])

                # ---- sort: Batcher merge-exchange on DVE, ping-pong with
                # 4x-mode carry copies for untouched runs (28 passes, even,
                # so the sorted result lands back in zA).
                cur, oth = zA, zB
                for pi, pp in enumerate(PASSES):
                    d, p, r = pp
                    nb = sum(1 for i in range(P - d) if (i & p) == r) // p
                    hi_i = sview(cur, r, 2 * p, nb, p, ng)
                    lo_i = sview(cur, r + d, 2 * p, nb, p, ng)
                    nc.vector.tensor_tensor(
                        out=sview(oth, r, 2 * p, nb, p, ng),
                        in0=hi_i, in1=lo_i, op=OP.max)
                    nc.vector.tensor_tensor(
                        out=sview(oth, r + d, 2 * p, nb, p, ng),
                        in0=hi_i, in1=lo_i, op=OP.min)
                    for (s0, stride, m, ln) in CARRIES[pi]:
                        if m > 1 and s0 + m * stride > P and s0 - (stride - ln) < 0:
                            for j in range(m):
                                nc.vector.tensor_copy(
                                    sview(oth, s0 + j * stride, ln, 1, ln, ng),
                                    sview(cur, s0 + j * stride, ln, 1, ln, ng))
                        else:
                            nc.vector.tensor_copy(
                                sview(oth, s0, stride, m, ln, ng),
                                sview(cur, s0, stride, m, ln, ng))
                    cur, oth = oth, cur

                # ---- pooling on Pool engine: rank-weight multiply, then a
                # log-tree fold over particles (fp16 until width 4, then f32).
                peng = nc.vector if st_i == len(CHUNKS) - 1 else nc.gpsimd
                prod = ppool.tile([128, P, NG], f16, tag="prod")
                peng.tensor_tensor(
                    out=prod[:, :, 0:ng], in0=cur[:, :, 0:ng],
                    in1=wrept_s[:, :, 0:ng], op=OP.mult)
                sf16 = fpool.tile([128, 64, NG], f16, tag="sf16")
                sf32 = fpool.tile([128, 2, NG], f32, tag="sf32")
                a, b = prod, sf16
                w = 64
                while w >= 4:
                    peng.tensor_tensor(
                        out=b[:, 0:w, 0:ng], in0=a[:, 0:w, 0:ng],
                        in1=a[:, w:2 * w, 0:ng], op=OP.add)
                    a, b = b, a
                    w //= 2
                # w folds left: a[:, 0:4] holds 4 partials -> f32 -> stage
                peng.tensor_tensor(
                    out=sf32[:, :, 0:ng], in0=a[:, 0:2, 0:ng],
                    in1=a[:, 2:4, 0:ng], op=OP.add)
                peng.tensor_tensor(
                    out=stage[:, col0:col0 + ng].rearrange("a (o s) -> a o s", o=1),
                    in0=sf32[:, 0:1, 0:ng], in1=sf32[:, 1:2, 0:ng], op=OP.add)

                # ---- epilogue on the stage layout itself: within each
                # 32-partition block, rows 0:16 are mus and 16:32 logvars;
                # eps is host-staged at the logvar rows of a [128,GALL]
                # tensor, so exp/mult/add run in place and the three output
                # DMAs use partition-grouped access patterns (no staging).
                csl = slice(col0, col0 + ng)
                eeng = nc.vector if st_i == len(CHUNKS) - 1 else nc.gpsimd
                eeng.tensor_scalar_add(
                    stage[:, csl], stage[:, csl], offs_s[:])
                # exp over all 128 rows (mus rows multiply to zero via the
                # host-zeroed eps at those rows); shift lv-row products down
                # 16 partitions via Pool-queue DMAs (keeps the SP queue free
                # for xt prefetch), add the mus rows in 32-aligned 16-row
                # ops, and emit raw-layout outputs that the host de-leaves.
                ex = epool.tile([128, ng], f32, tag="ex")
                nc.scalar.activation(ex[:], stage[:, csl], AF.Exp, scale=0.5)
                eeng.tensor_tensor(
                    out=ex[:], in0=ex[:], in1=epst_s[:, csl], op=OP.mult)
                last = st_i == len(CHUNKS) - 1
                smpv = epool.tile([128, ng], f32, tag="smpv")
                qengs = ([nc.sync, nc.gpsimd, nc.scalar, nc.sync]
                         if last else [nc.gpsimd] * 4)
                for q in range(4):
                    # at the drain all queues are empty: spread the shift
                    # DMAs so their serial latency collapses
                    qengs[q].dma_start(
                        out=smpv[32 * q:32 * q + 16, :],
                        in_=ex[32 * q + 16:32 * q + 32, :])
                for q in range(4):
                    eeng.tensor_tensor(
                        out=smpv[32 * q:32 * q + 16, :],
                        in0=smpv[32 * q:32 * q + 16, :],
                        in1=stage[32 * q:32 * q + 16, csl], op=OP.add)
                (nc.sync if last else nc.gpsimd).dma_start(
                    out=stage_d[:, csl], in_=stage[:, csl])
                nc.gpsimd.dma_start(out=smp_d[:, csl], in_=smpv[:])

                ev0 += st_e

    nc.compile()
    _BUILT = nc
    return nc


def _host_prep(x, W1, b1, W2, b2, W3, b3, pool_weight, eps):
    x = np.asarray(x, np.float32)
    eps = np.asarray(eps, np.float32)
    W1 = np.asarray(W1, np.float32).astype(np.float16)
    W2 = np.asarray(W2, np.float32).astype(np.float16)
    W3 = np.asarray(W3, np.float32)
    b1 = np.asarray(b1, np.float32).reshape(H, 1)
    b2 = np.asarray(b2, np.float32).reshape(H, 1)
    b3 = np.asarray(b3, np.float32)
    pw = np.asarray(pool_weight, np.float32)

    # channel permutation: device channel c' maps to logical channel perm[c']
    # (mus channels 0,2,..,30 first, then logvar channels 1,3,..,31)
    perm = np.concatenate([np.arange(0, C, 2), np.arange(1, C, 2)])
    W3 = np.ascontiguousarray(W3[:, perm]).astype(np.float16)
    b3p = b3[perm]
    w_table = (pw @ _fspool_interp_matrix()).astype(np.float32)[perm]  # [32, 128]
    wrep = np.tile(w_table, (4, 1))                                    # [128, 128]
    wrept = np.ascontiguousarray(
        np.broadcast_to(wrep[:, :, None], (128, P, NG))
    ).astype(np.float16)
    offs = np.tile(b3p * w_table.sum(axis=1), 4).reshape(128, 1).astype(np.float32)

    in_maps = []
    for c in range(NCORES):
        xs = x[c * E:(c + 1) * E]                                  # [E, 512]
        xt = np.ascontiguousarray(
            xs.reshape(E, P, F).transpose(2, 0, 1).reshape(F, E * P)
        ).astype(np.float16)
        es = eps[c * E:(c + 1) * E]                                # [E, 16]
        ep64 = es.reshape(GALL, 4, LAT).transpose(1, 2, 0).reshape(64, GALL)
        epst = np.zeros((128, GALL), np.float32)
        for q in range(4):
            epst[32 * q + 16:32 * q + 32] = ep64[16 * q:16 * (q + 1)]
        in_maps.append({
            "xt": xt, "w1": W1, "w2": W2, "w3": W3,
            "b1": b1, "b2": b2, "wrept": wrept, "offs": offs, "epst": epst,
        })
    return in_maps


def _host_post(results):
    mus = np.empty((B, LAT), np.float32)
    logvars = np.empty((B, LAT), np.float32)
    samples = np.empty((B, LAT), np.float32)
    for c, r in enumerate(results):
        st = r["stage_t"].reshape(4, 2, LAT, GALL)    # [e4, mus/lv, lat, g]
        sm = r["samples_t"].reshape(4, 2, LAT, GALL)
        for src_, dst in ((st[:, 0], mus), (st[:, 1], logvars),
                          (sm[:, 0], samples)):
            dst[c * E:(c + 1) * E] = (
                src_.transpose(2, 0, 1).reshape(E, LAT))
    return mus, logvars, samples


def kernel(**inputs):
    global LAST_RESULTS
    from concourse.bass_utils import run_bass_kernel_spmd

    nc = _build()
    in_maps = _host_prep(**inputs)
    trace = bool(int(os.environ.get("KERNEL_TRACE", "0")))
    res = run_bass_kernel_spmd(nc, in_maps, list(range(NCORES)), trace=trace)
    LAST_RESULTS = res
    return _host_post(res.results)
